# revision 93
# speedup vs baseline: 2.1587x; 1.0040x over previous
"""Trainium2 Bass kernel for nn_EnhancedReflectiveCognitiveGraph (GNN edge-softmax attention).

Math (see reference):
  q/k/v = x @ W{q,k,v}.T + b ; per-edge scores s_e = <q[src_e], k[dest_e]>_head / 4
  softmax over edges sharing src (max-subtraction skipped: scores ~ N(0,1) so
  exp never overflows in fp16 and the weights are mathematically identical;
  the 1/sqrt(d) scale is folded into Wq/bq on the host)
  agg[dest] += exp_e * recip[src_e] * v[src_e] ; out = agg @ Wo.T + bo

Device strategy (8 cores, node-range sharding, three SPMD NEFF launches):
  L1 (proj): each core computes q/k/v (fp16) for its node shard; the host
      keeps the full q/k/v tables (pure relayout).
  L2 (src phase): core c owns edges with src in its shard, laid out in
      128-edge chunks grouped per src block (per-block chunk count k[b] =
      max over cores, so one program serves all 8; the host bin-packs nodes
      into blocks to minimize chunk padding).  q and k rows arrive as
      host-pre-gathered contiguous streams (qexp/kexp, full-bandwidth DMA —
      the host gather is pure relayout of device-computed tables); per-edge
      scores via one fp16 2x multiply + a tree of fp16 adds over the head
      dim; exp (Act); per-src-node segment sums via PE matmuls with one-hot
      matrices (S^T, mostly streamed fp8, a tuned fraction built on DVE via
      iota/is_equal); clamped reciprocal.  Outputs per-edge exp and
      per-node recip.
  L3 (dest phase): core c owns edges with dest in its shard.  v rows arrive
      host-pre-gathered (vexp); the host also permutes exp and gathers recip
      rows per edge (relayout only) so the device computes the softmax
      weights w = exp * recip (fp16 2x) and wv = w * v, scatter-added into
      per-dest-block agg via PE matmuls with streamed one-hots (T^T), then
      the output projection.  agg is complete locally (dest-sharded):
      no collectives and no racy HBM scatter-adds anywhere.
  Host between launches does pure relayout (gather/permute/pad/zero/cast-up).
"""

import math
import ml_dtypes
import numpy as np

import concourse.bacc as bacc
import concourse.mybir as mybir
import concourse.tile as tile
from concourse.bass_utils import run_bass_kernel_spmd

# ---------------------------------------------------------------- constants
N = 50000
E = 600000
F = 128
H = 8
Dh = 16
P = 128
C = 8                     # cores
SH = 6272                 # nodes per core, cores 0-6 (49 blocks); core 7: 6096
NB = 49                   # blocks per shard (core 7 block 48 is partial)
SBV2 = 32                 # chunks per value-stream DMA batch, L2 (fp16)
SBV3 = 36                 # chunks per value-stream DMA batch, L3 (fp16)
SBO = 64                  # chunks per one-hot stream DMA batch (fp8)
SBO2 = 32                 # chunks per one-hot batch when hybrid (stream/build)
OH_BUILD_L2 = 0.0         # fraction of one-hot batches built on DVE in L2
OH_BUILD_L3 = 0.15        # fraction built on DVE in L3
POOL_MULT_L2 = 0.0        # fraction of L2 mult blocks on Pool
POOL_MULT_L3 = 0.0        # fraction of L3 wv-mult blocks on Pool
QS_BLOCKS = 44            # L2 blocks whose q is expanded on-chip (prefix)
QE_CB = 6                 # chunks per qe PSUM batch in the on-chip q path
ESPL = 12                 # blocks per exp/rec output write group
F16 = mybir.dt.float16
F8 = mybir.dt.float8e4
F32 = mybir.dt.float32


def shard_base(c):
    return c * SH


def shard_len(c):
    return min(N, (c + 1) * SH) - c * SH


# ---------------------------------------------------------------- host prep
class ChunkMap:
    """Uniform chunk structure shared by all cores for one phase.

    Chunks (128 slots each) are grouped per key-block: k[b] chunks for block
    b, sized so every core's edges fit.  chunk -> block is data-independent;
    only slot contents differ per core."""

    def __init__(self, ks):
        self.ks = list(ks)
        self.chunks = [b for b in range(NB) for _ in range(self.ks[b])]
        self.block_start = np.cumsum([0] + self.ks).tolist()
        self.nch = len(self.chunks)
        self.nslots = self.nch * P
        self.kmax = max(self.ks)


def _pack_blocks(deg):
    """Pack nodes (by per-phase degree) into NB blocks of <=128 nodes so all
    but a few overflow blocks stay under 12*128 edges: snake-deal by degree
    (near-equal sums), then swap high-degree nodes into the overflow blocks.
    Returns (block, pos) per node and per-block edge sums."""
    ln = len(deg)
    order = np.argsort(-deg, kind="stable")
    members = [[] for _ in range(NB)]
    i = 0
    fwd = True
    while i < ln:
        for b in (range(NB) if fwd else range(NB - 1, -1, -1)):
            if i >= ln:
                break
            members[b].append(order[i])
            i += 1
        fwd = not fwd
    for b in range(NB):
        members[b].sort(key=lambda n: -deg[n])   # desc degree within block
    sums = np.array([int(deg[m].sum()) if len(m) else 0
                     for m in [np.array(mm, np.int64) for mm in members]])
    target = 12 * P
    excess = int(deg.sum()) - NB * target
    m_over = (excess + P - 1) // P + 1 if excess > 0 else 0
    if m_over:
        over = list(np.argsort(sums)[-m_over:])
        over_set = set(over)
        for b in range(NB):
            if b in over_set:
                continue
            while sums[b] > target:
                u = members[b][0]          # largest-degree node in b
                best = None
                for o in over:
                    v = members[o][-1]     # smallest-degree node in o
                    if deg[v] < deg[u] and (best is None
                                            or sums[o] < sums[best[0]]):
                        best = (o, v)      # fill the emptiest overflow block
                if best is None:
                    break
                o, v = best
                members[b].remove(u)
                members[o].remove(v)
                members[b].append(v)
                members[o].insert(0, u)
                members[b].sort(key=lambda n: -deg[n])
                sums[b] += deg[v] - deg[u]
                sums[o] += deg[u] - deg[v]
    # order blocks by edge sum so chunk counts align across cores
    order_b = np.argsort(sums, kind="stable")
    blk_of = np.zeros(ln, np.int64)
    pos = np.zeros(ln, np.int64)
    for newb, b in enumerate(order_b):
        idx = np.array(members[b], np.int64)
        blk_of[idx] = newb
        pos[idx] = np.arange(len(idx))
    return blk_of, pos, sums[order_b]


def compute_cmap(key, other):
    """Per-block chunk counts (max over cores) for one phase, with host-side
    node->block packing (pure relabeling) to minimize chunk padding."""
    need = np.ones(NB, dtype=np.int64)
    perm_block, perm_pos = [], []
    for c in range(C):
        base, ln = shard_base(c), shard_len(c)
        m = (key >= base) & (key < base + ln)
        deg = np.bincount(key[m] - base, minlength=ln)[:ln]
        blk_of, pos, sums = _pack_blocks(deg)
        perm_block.append(blk_of)
        perm_pos.append(pos)
        need = np.maximum(need, (sums + P - 1) // P)
    cm = ChunkMap(need.tolist())
    cm.perm_block = perm_block
    cm.perm_pos = perm_pos
    return cm


class CorePlan:
    """Per-core slot contents for one phase.  `key` = node defining the block
    (src for L2, dest for L3); `other` = the opposite endpoint (indexes the
    host-side gather tables)."""

    def __init__(self, cmap, core, key, other, edge_ids):
        base = shard_base(core)
        pb = cmap.perm_block[core]
        pp = cmap.perm_pos[core]
        self.slot_local = np.full(cmap.nslots, -1, np.int64)
        self.slot_key = np.zeros(cmap.nslots, np.int64)
        self.slot_gidx = np.zeros(cmap.nslots, np.int64)
        self.slot_edge = np.full(cmap.nslots, -1, np.int64)
        block = pb[key - base]
        loc = pp[key - base]
        for b in range(NB):
            m = block == b
            cnt = int(m.sum())
            if cnt == 0:
                continue
            assert cnt <= cmap.ks[b] * P
            s0 = cmap.block_start[b] * P
            self.slot_local[s0:s0 + cnt] = loc[m]
            self.slot_key[s0:s0 + cnt] = key[m]
            self.slot_gidx[s0:s0 + cnt] = other[m]
            self.slot_edge[s0:s0 + cnt] = edge_ids[m]
        self.cmap = cmap

    def onehot_stream(self, transposed=True):
        """One-hot [128, nch*128] fp8; chunk c at cols c*128:(c+1)*128.
        transposed=True: S^T [slot(p), key_local] (seg-sum lhsT);
        transposed=False: S [key_local(p), slot] (expansion lhsT).
        Dummy slots are all-zero rows/columns."""
        cm = self.cmap
        out = np.zeros((P, cm.nch * P), dtype=ml_dtypes.float8_e4m3)
        loc = self.slot_local
        sl_all = np.arange(cm.nslots)
        valid = loc >= 0
        ch = sl_all // P
        row = sl_all % P
        if transposed:
            out[row[valid], ch[valid] * P + loc[valid]] = 1.0
        else:
            out[loc[valid], ch[valid] * P + row[valid]] = 1.0
        return out

    def expand_rows(self, table, idx):
        """Host gather: [128, nch*rowlen] stream, slot (c, p) holds
        table[idx[c*128+p]] (pure relayout of a device-computed table)."""
        cm = self.cmap
        rowlen = table.shape[1]
        g = table[idx.reshape(cm.nch, P)]          # [nch, P, rowlen]
        return np.ascontiguousarray(
            g.transpose(1, 0, 2).reshape(P, cm.nch * rowlen))

    def loc_idx(self):
        """[128, nch] int16: slot_local per (slot-partition, chunk); -1 for
        dummy slots (matches nothing when one-hots are built on-chip)."""
        return np.ascontiguousarray(
            self.slot_local.reshape(self.cmap.nch, P).T.astype(np.int16))


# ---------------------------------------------------------------- L1: q/k/v projections
def build_l1(with_bias=False):
    nc = bacc.Bacc("TRN2", target_bir_lowering=False, num_devices=C)
    xT = nc.dram_tensor("xT", [P, NB * P], F16, kind="ExternalInput")
    wqkv = nc.dram_tensor("wqkv", [P, 3 * P], F16, kind="ExternalInput")
    bqkv = nc.dram_tensor("bqkv", [P, 3], F16, kind="ExternalInput")
    # q/k/v transposed [f_out, node], interleaved per block; the host
    # de-interleaves and transposes (pure relayout)
    qkv_sh = nc.dram_tensor("qkv_sh", [P, NB * 3 * P], F16,
                            kind="ExternalOutput")

    with tile.TileContext(nc) as tc:
        with tc.tile_pool(name="const", bufs=1) as cpool, \
             tc.tile_pool(name="psum", bufs=4, space="PSUM") as ppool:
            w_sb = cpool.tile([P, 3 * P], F16, tag="w")
            nc.sync.dma_start(w_sb[:], wqkv[:])
            b_sb = cpool.tile([P, 3], F16, tag="b")
            nc.sync.dma_start(b_sb[:], bqkv[:])
            xt = cpool.tile([P, NB * P], F16, tag="xT")
            LD = 7

            def load_x(slice_i):
                b0 = slice_i * LD
                if b0 >= NB:
                    return
                n = min(LD, NB - b0) * P
                nc.gpsimd.dma_start(xt[:, b0 * P:b0 * P + n],
                                    xT[:, b0 * P:b0 * P + n])

            # two slices ahead; the rest interleave with output writes so
            # the serial DMA engine alternates input/output instead of
            # front-loading all input
            load_x(0)
            load_x(1)
            osb = cpool.tile([P, NB * 3 * P], F16, tag="osb")
            for b in range(NB):
                if b % LD == 0 and b > 0:
                    load_x(b // LD + 1)
                # out[f_out, node] = W[f_in, f_out]^T @ xT[f_in, node]:
                # bias becomes a per-partition scalar, no bias matmul
                ps = ppool.tile([P, 3 * P], F32, tag="proj")
                for t in range(3):
                    nc.tensor.matmul(ps[:, t * P:(t + 1) * P],
                                     lhsT=w_sb[:, t * P:(t + 1) * P],
                                     rhs=xt[:, b * P:(b + 1) * P],
                                     start=True, stop=True)
                    if with_bias:
                        dst = osb[:, (b * 3 + t) * P:(b * 3 + t + 1) * P]
                        if (b * 3 + t) % 2:
                            nc.vector.tensor_tensor(
                                out=dst, in0=ps[:, t * P:(t + 1) * P],
                                in1=b_sb[:, t:t + 1].broadcast_to([P, P]),
                                op=mybir.AluOpType.add)
                        else:
                            nc.scalar.activation(
                                out=dst, in_=ps[:, t * P:(t + 1) * P],
                                func=mybir.ActivationFunctionType.Identity,
                                bias=b_sb[:, t:t + 1])
                if not with_bias:
                    # biases are zero: one whole-block copy, alternating
                    (nc.vector.tensor_copy if b % 2 else nc.scalar.copy)(
                        osb[:, b * 3 * P:(b + 1) * 3 * P], ps[:])
                # write in 4-block groups from the Pool queue so output DMA
                # overlaps compute from block 3 onward
                WD = 4
                if b % WD == WD - 1 or b == NB - 1:
                    b0 = b // WD * WD
                    n = (b - b0 + 1) * 3 * P
                    nc.sync.dma_start(qkv_sh[:, b0 * 3 * P:b0 * 3 * P + n],
                                      osb[:, b0 * 3 * P:b0 * 3 * P + n])
    nc.compile()
    return nc


# ---------------------------------------------------------------- streaming helper
def make_streamer(nc, spool, nch, sb, first=None, last=None, lstep=8):
    """Stream [P, nch*rowlen] dram in sb-chunk batches.  A smaller first
    batch (`first`) shortens the pipeline ramp; splitting the final `last`
    chunks into `lstep`-sized batches shortens the drain tail."""
    bounds = [0]
    x = 0
    while x < nch:
        if x == 0 and first:
            step = first
        elif last and x >= nch - last:
            step = lstep
        else:
            step = sb
        x = min(nch, x + step)
        bounds.append(x)

    def stream_tile(tiles, dram, ci, dt, rowlen=P):
        import bisect
        i = bisect.bisect_right(bounds, ci) - 1
        b0, hi = bounds[i], bounds[i + 1]
        if b0 not in tiles:
            t = spool.tile([P, sb * rowlen], dt, tag=dram.name,
                           name=f"strm_{dram.name}_{b0}")
            n = (hi - b0) * rowlen
            nc.sync.dma_start(t[:, :n], dram[:, b0 * rowlen:b0 * rowlen + n])
            tiles[b0] = t
        return tiles[b0], b0, hi
    return stream_tile


def make_onehot_source(nc, pool, nch, sb, dram, idx_sb, iota_t, build_frac):
    """One-hot chunk source: batches are either DMA-streamed from `dram` or
    built on DVE (is_equal is not in the Pool engine's ISA) from compact
    local indices via iota/is_equal.  `build_frac` of batches are built,
    spread evenly."""
    tiles = {}
    nbatch = (nch + sb - 1) // sb
    built = set(i for i in range(nbatch)
                if int(i * build_frac) != int((i + 1) * build_frac))

    def get(ci):
        b0 = ci // sb * sb
        hi = min(b0 + sb, nch)
        if b0 not in tiles:
            n = hi - b0
            t = pool.tile([P, sb * P], F8, tag=dram.name,
                          name=f"oh_{dram.name}_{b0}")
            if (b0 // sb) in built:
                nc.vector.tensor_tensor(
                    out=t[:, :n * P].rearrange("p (c q) -> p c q", q=P),
                    in0=iota_t[:, None, :].broadcast_to([P, n, P]),
                    in1=idx_sb[:, b0:hi][:, :, None].broadcast_to([P, n, P]),
                    op=mybir.AluOpType.is_equal)
            else:
                nc.sync.dma_start(t[:, :n * P], dram[:, b0 * P:hi * P])
            tiles[b0] = t
        return tiles[b0], b0, hi
    return get


# ---------------------------------------------------------------- L2: src phase
def build_l2(cmap, qs_blocks=None):
    """qs_blocks: the first qs_blocks blocks expand q ON-CHIP (PE one-hot
    matmul from a resident per-shard q table + Act PSUM->fp16 copy) instead
    of reading the host-expanded qexp stream — trades idle Act/PE cycles for
    DMA bytes."""
    if qs_blocks is None:
        qs_blocks = QS_BLOCKS
    nch, kmax = cmap.nch, cmap.kmax
    nc = bacc.Bacc("TRN2", target_bir_lowering=False, num_devices=C)
    qexp = nc.dram_tensor("qexp", [P, nch * P], F16, kind="ExternalInput")
    kexp = nc.dram_tensor("kexp", [P, nch * P], F16, kind="ExternalInput")
    q_sh = nc.dram_tensor("q_sh", [P, NB * P], F16, kind="ExternalInput")
    S_st = nc.dram_tensor("S_st", [P, nch * P], F8, kind="ExternalInput")
    ST_st = nc.dram_tensor("ST_st", [P, nch * P], F8, kind="ExternalInput")
    st_idx = nc.dram_tensor("st_idx", [P, nch], mybir.dt.int16,
                            kind="ExternalInput")
    exp_out = nc.dram_tensor("exp_out", [P, nch * H], F16, kind="ExternalOutput")
    rec_out = nc.dram_tensor("rec_out", [P, NB * H], F16, kind="ExternalOutput")
    CB = QE_CB                 # chunks per qe PSUM batch (2 banks x2 bufs)

    with tile.TileContext(nc) as tc:
        with tc.tile_pool(name="resident", bufs=1) as rpool, \
             tc.tile_pool(name="ostream", bufs=3) as spool, \
             tc.tile_pool(name="vstream", bufs=4) as vpool, \
             tc.tile_pool(name="work", bufs=3) as wpool, \
             tc.tile_pool(name="seg_psum", bufs=2, space="PSUM") as gpsum, \
             tc.tile_pool(name="qe_psum", bufs=2, space="PSUM") as qpsum:
            exp_sb = rpool.tile([P, nch * H], F16, tag="exp_sb")
            rec_sb = rpool.tile([P, NB * H], F16, tag="rec_sb")
            idx_sb = iota_t = None
            if OH_BUILD_L2 > 0:
                idx_sb = rpool.tile([P, nch], mybir.dt.int16, tag="idx_sb")
                nc.sync.dma_start(idx_sb[:], st_idx[:])
                iota_t = rpool.tile([P, P], mybir.dt.int16, tag="iota")
                nc.gpsimd.iota(iota_t[:], pattern=[[1, P]], base=0,
                               channel_multiplier=0)
            st_src = make_onehot_source(nc, spool, nch, SBO2, ST_st,
                                        idx_sb, iota_t, OH_BUILD_L2)
            vstream = make_streamer(nc, vpool, nch, SBV2, first=8)
            sstream = make_streamer(nc, spool, nch, SBO2)
            q_tiles, k_tiles, s_tiles = {}, {}, {}
            qsb = None
            if qs_blocks:
                qsb = rpool.tile([P, NB * P], F16, tag="q_sh_sb")
                # staged load: early blocks' slices land first so the qe
                # pipeline starts immediately
                edges = [0, 2, 6, 14, 28, qs_blocks]
                for lo, hi in zip(edges, edges[1:]):
                    hi = min(hi, qs_blocks)
                    if hi > lo:
                        # Pool queue: interleaves with SP-queue stream loads
                        nc.gpsimd.dma_start(qsb[:, lo * P:hi * P],
                                            q_sh[:, lo * P:hi * P])

            esplit = [cmap.block_start[min(b0 + ESPL, NB)]
                      for b0 in range(0, NB, ESPL)]

            for b in range(NB):
                kb = cmap.ks[b]
                s0 = cmap.block_start[b]
                qk = wpool.tile([P, kmax * P], F16, tag="qk")
                if b < qs_blocks:
                    # on-chip q expansion: PE matmuls + scaled Act copy
                    qe16 = wpool.tile([P, kmax * P], F16, tag="qe16")
                    ci = s0
                    while ci < s0 + kb:
                        cn = min(CB, s0 + kb - ci)
                        qe_ps = qpsum.tile([P, CB * P], F32, tag="qe",
                                           name=f"qe_{ci}")
                        for j in range(cn):
                            st, sb0, _ = sstream(s_tiles, S_st, ci + j, F8)
                            off = ci + j - sb0
                            nc.tensor.matmul(
                                qe_ps[:, j * P:(j + 1) * P],
                                lhsT=st[:, off * P:(off + 1) * P],
                                rhs=qsb[:, b * P:(b + 1) * P],
                                start=True, stop=True)
                        nc.scalar.activation(
                            out=qe16[:, (ci - s0) * P:(ci - s0 + cn) * P],
                            in_=qe_ps[:, :cn * P],
                            func=mybir.ActivationFunctionType.Copy,
                            scale=1.0)
                        ci += cn
                # qk = q * kexp (DVE 2x), split at stream-batch boundaries
                ci = s0
                while ci < s0 + kb:
                    kt, b0, bhi = vstream(k_tiles, kexp, ci, F16)
                    cj = min(s0 + kb, bhi)
                    n = cj - ci
                    if b < qs_blocks:
                        nc.vector.tensor_mul(
                            qk[:, (ci - s0) * P:(ci - s0 + n) * P],
                            qe16[:, (ci - s0) * P:(ci - s0 + n) * P],
                            kt[:, (ci - b0) * P:(ci - b0 + n) * P])
                    else:
                        qt, qb0, _ = vstream(q_tiles, qexp, ci, F16)
                        nc.vector.tensor_mul(
                            qk[:, (ci - s0) * P:(ci - s0 + n) * P],
                            qt[:, (ci - qb0) * P:(ci - qb0 + n) * P],
                            kt[:, (ci - b0) * P:(ci - b0 + n) * P])
                    ci = cj
                # tree-reduce over head dim (DVE 2x fp16 adds)
                a1 = wpool.tile([P, kmax * P // 2], F16, tag="a1")
                qk4 = qk[:, :kb * P].rearrange("p (c h d) -> p c h d", h=H, d=Dh)
                nc.vector.tensor_add(
                    out=a1[:, :kb * P // 2].rearrange("p (c h d) -> p c h d", h=H, d=8),
                    in0=qk4[:, :, :, 0:8], in1=qk4[:, :, :, 8:16])
                a2 = wpool.tile([P, kmax * P // 4], F16, tag="a2")
                a14 = a1[:, :kb * P // 2].rearrange("p (c h d) -> p c h d", h=H, d=8)
                nc.vector.tensor_add(
                    out=a2[:, :kb * P // 4].rearrange("p (c h d) -> p c h d", h=H, d=4),
                    in0=a14[:, :, :, 0:4], in1=a14[:, :, :, 4:8])
                a3 = wpool.tile([P, kmax * P // 8], F16, tag="a3")
                a24 = a2[:, :kb * P // 4].rearrange("p (c h d) -> p c h d", h=H, d=4)
                nc.vector.tensor_add(
                    out=a3[:, :kb * P // 8].rearrange("p (c h d) -> p c h d", h=H, d=2),
                    in0=a24[:, :, :, 0:2], in1=a24[:, :, :, 2:4])
                sc = wpool.tile([P, kmax * H], F16, tag="sc")
                a34 = a3[:, :kb * P // 8].rearrange("p (c h d) -> p c h d", h=H, d=2)
                nc.vector.tensor_add(
                    out=sc[:, :kb * H].rearrange("p (c h) -> p c h", h=H)[:, :, :, None],
                    in0=a34[:, :, :, 0:1], in1=a34[:, :, :, 1:2])
                # exp (Act) straight into the resident output tile
                nc.scalar.activation(
                    out=exp_sb[:, s0 * H:(s0 + kb) * H], in_=sc[:, :kb * H],
                    func=mybir.ActivationFunctionType.Exp, scale=1.0)
                # segment sums over slots per src node (PE, accumulate)
                seg_ps = gpsum.tile([P, H], F32, tag="seg")
                for j in range(kb):
                    st, sb0, _ = st_src(s0 + j)
                    off = s0 + j - sb0
                    nc.tensor.matmul(
                        seg_ps[:],
                        lhsT=st[:, off * P:(off + 1) * P],
                        rhs=exp_sb[:, (s0 + j) * H:(s0 + j + 1) * H],
                        start=(j == 0), stop=(j == kb - 1))
                # clamped reciprocal: zero-degree rows give a finite value
                # (cancelled downstream); real rows have seg >> 1e-4
                s2 = wpool.tile([P, H], F32, tag="s2")
                nc.vector.tensor_scalar_max(s2[:], seg_ps[:], 1e-4)
                rec_raw = wpool.tile([P, H], F32, tag="rec_raw")
                nc.vector.reciprocal(rec_raw[:], s2[:])
                nc.scalar.copy(rec_sb[:, b * H:(b + 1) * H], rec_raw[:])
                # stream finished exp/rec columns out in 6-block groups
                hi = cmap.block_start[b + 1]
                if hi in esplit:
                    lo = esplit[esplit.index(hi) - 1] if esplit.index(hi) else 0
                    nc.gpsimd.dma_start(exp_out[:, lo * H:hi * H],
                                        exp_sb[:, lo * H:hi * H])
                    lob = b // ESPL * ESPL
                    nc.gpsimd.dma_start(rec_out[:, lob * H:(b + 1) * H],
                                        rec_sb[:, lob * H:(b + 1) * H])
    nc.compile()
    return nc


# ---------------------------------------------------------------- L3: dest phase
def build_l3(cmap):
    nch, kmax = cmap.nch, cmap.kmax
    nc = bacc.Bacc("TRN2", target_bir_lowering=False, num_devices=C)
    vexp = nc.dram_tensor("vexp", [P, nch * P], F16, kind="ExternalInput")
    TT_st = nc.dram_tensor("TT_st", [P, nch * P], F8, kind="ExternalInput")
    tt_idx = nc.dram_tensor("tt_idx", [P, nch], mybir.dt.int16,
                            kind="ExternalInput")
    exp_in = nc.dram_tensor("exp_in", [P, nch * H], F16, kind="ExternalInput")
    rexp_in = nc.dram_tensor("rexp_in", [P, nch * H], F16, kind="ExternalInput")
    WoT = nc.dram_tensor("WoT", [P, P], F16, kind="ExternalInput")
    bo_r = nc.dram_tensor("bo_r", [1, P], F16, kind="ExternalInput")
    ones = nc.dram_tensor("ones", [1, P], F16, kind="ExternalInput")
    outT = nc.dram_tensor("outT", [P, NB * P], F16, kind="ExternalOutput")

    with tile.TileContext(nc) as tc:
        with tc.tile_pool(name="resident", bufs=1) as rpool, \
             tc.tile_pool(name="ostream", bufs=3) as spool, \
             tc.tile_pool(name="vstream", bufs=4) as vpool, \
             tc.tile_pool(name="work", bufs=3) as wpool, \
             tc.tile_pool(name="agg_psum", bufs=2, space="PSUM") as apsum, \
             tc.tile_pool(name="out_psum", bufs=2, space="PSUM") as opsum:
            idx_sb = iota_t = None
            if OH_BUILD_L3 > 0:
                idx_sb = rpool.tile([P, nch], mybir.dt.int16, tag="idx_sb")
                nc.sync.dma_start(idx_sb[:], tt_idx[:])
                iota_t = rpool.tile([P, P], mybir.dt.int16, tag="iota")
                nc.gpsimd.iota(iota_t[:], pattern=[[1, P]], base=0,
                               channel_multiplier=0)
            tt_src = make_onehot_source(nc, spool, nch, SBO2, TT_st,
                                        idx_sb, iota_t, OH_BUILD_L3)
            vstream = make_streamer(nc, vpool, nch, SBV3, first=8, last=24)
            v_tiles = {}
            # prefetch the first value/one-hot batches before anything else
            vstream(v_tiles, vexp, 0, F16)
            tt_src(0)
            w_all = rpool.tile([P, nch * H], F16, tag="w_all")
            for lo_b, hi_b in ((0, 6), (6, 16), (16, 27), (27, 38), (38, NB)):
                lo, hi = cmap.block_start[lo_b], cmap.block_start[hi_b]
                nc.vector.tensor_mul(
                    w_all[:, lo * H:hi * H],
                    nc_stream_slice(nc, exp_in, rpool, lo, hi),
                    nc_stream_slice(nc, rexp_in, rpool, lo, hi))
            wo_sb = rpool.tile([P, P], F16, tag="wo")
            nc.sync.dma_start(wo_sb[:], WoT[:])
            bo_sb = rpool.tile([1, P], F16, tag="bo")
            nc.sync.dma_start(bo_sb[:], bo_r[:])
            ones_sb = rpool.tile([1, P], F16, tag="ones")
            nc.sync.dma_start(ones_sb[:], ones[:])
            osb = rpool.tile([P, NB * P], F16, tag="osb")

            for b in range(NB):
                kb = cmap.ks[b]
                s0 = cmap.block_start[b]
                # wv = w * v: per-block strategy balances Pool/Act/DVE.
                # 'act': Act broadcasts w, DVE multiplies in 2x mode;
                # 'dve': single DVE 1x broadcast-multiply;
                # 'pool': single Pool broadcast-multiply.
                if int(b * POOL_MULT_L3) != int((b + 1) * POOL_MULT_L3):
                    strat = 'pool'
                else:
                    strat = 'act' if b % 2 == 0 else 'dve'
                wv = wpool.tile([P, kmax * P], F16, tag="wv")
                if strat == 'act':
                    # Act broadcast of w, then DVE 2x multiply
                    erep = wpool.tile([P, kmax * P], F16, tag="erep")
                    nc.scalar.copy(
                        erep[:, :kb * P].rearrange("p (c h d) -> p c h d", h=H, d=Dh),
                        w_all[:, s0 * H:(s0 + kb) * H]
                        .rearrange("p (c h) -> p c h", h=H)[:, :, :, None]
                        .broadcast_to([P, kb, H, Dh]))
                ci = s0
                while ci < s0 + kb:
                    vt, b0, bhi = vstream(v_tiles, vexp, ci, F16)
                    cj = min(s0 + kb, bhi)
                    n = cj - ci
                    if strat == 'act':
                        nc.vector.tensor_mul(
                            wv[:, (ci - s0) * P:(ci - s0 + n) * P],
                            vt[:, (ci - b0) * P:(ci - b0 + n) * P],
                            erep[:, (ci - s0) * P:(ci - s0 + n) * P])
                    else:
                        eng = (nc.gpsimd.tensor_tensor if strat == 'pool'
                               else nc.vector.tensor_tensor)
                        eng(out=wv[:, (ci - s0) * P:(ci - s0 + n) * P]
                            .rearrange("p (c h d) -> p c h d", h=H, d=Dh),
                            in0=vt[:, (ci - b0) * P:(ci - b0 + n) * P]
                            .rearrange("p (c h d) -> p c h d", h=H, d=Dh),
                            in1=w_all[:, ci * H:(ci + n) * H]
                            .rearrange("p (c h) -> p c h", h=H)[:, :, :, None]
                            .broadcast_to([P, n, H, Dh]),
                            op=mybir.AluOpType.mult)
                    ci = cj
                # scatter-add into aggT[f, dest_local] (PE, accumulate)
                agg_ps = apsum.tile([P, P], F32, tag="agg")
                for j in range(kb):
                    tt, sb0, _ = tt_src(s0 + j)
                    off = s0 + j - sb0
                    nc.tensor.matmul(
                        agg_ps[:],
                        lhsT=wv[:, j * P:(j + 1) * P],
                        rhs=tt[:, off * P:(off + 1) * P],
                        start=(j == 0), stop=(j == kb - 1))
                agg16 = wpool.tile([P, P], F16, tag="agg16")
                nc.scalar.copy(agg16[:], agg_ps[:])
                # output projection (PE) and fp16 store
                ops = opsum.tile([P, P], F32, tag="outp")
                nc.tensor.matmul(ops[:], lhsT=wo_sb[:], rhs=agg16[:],
                                 start=True, stop=False)
                nc.tensor.matmul(ops[:], lhsT=bo_sb[:], rhs=ones_sb[:],
                                 start=False, stop=True)
                nc.scalar.copy(osb[:, b * P:(b + 1) * P], ops[:])
                if b % 7 == 6 or b == NB - 1:
                    b0 = b // 7 * 7
                    n = (b - b0 + 1) * P
                    nc.gpsimd.dma_start(outT[:, b0 * P:b0 * P + n],
                                        osb[:, b0 * P:b0 * P + n])
    nc.compile()
    return nc


def nc_stream_slice(nc, dram, pool, lo, hi):
    """Load a column slice of a small dram tensor into a fresh tile."""
    t = pool.tile([P, (hi - lo) * H], F16, tag=f"{dram.name}_{lo}",
                  name=f"{dram.name}_{lo}")
    nc.sync.dma_start(t[:], dram[:, lo * H:hi * H])
    return t[:]


# ---------------------------------------------------------------- orchestration
def kernel(node_features, edge_index, Wq, bq, Wk, bk, Wv, bv, Wo, bo):
    node_features = np.asarray(node_features, np.float32)
    edge_index = np.asarray(edge_index)
    src, dst = edge_index[0].astype(np.int64), edge_index[1].astype(np.int64)
    x16 = node_features.astype(np.float16)
    s = 1.0 / math.sqrt(Dh)   # fold score scale into Wq/bq (parameter prep)
    w16 = {"Wq": (np.asarray(Wq, np.float32) * s).astype(np.float16),
           "Wk": np.asarray(Wk, np.float32).astype(np.float16),
           "Wv": np.asarray(Wv, np.float32).astype(np.float16),
           "Wo": np.asarray(Wo, np.float32).astype(np.float16)}
    b16 = {"bq": (np.asarray(bq, np.float32) * s).astype(np.float16),
           "bk": np.asarray(bk, np.float32).astype(np.float16),
           "bv": np.asarray(bv, np.float32).astype(np.float16),
           "bo": np.asarray(bo, np.float32).astype(np.float16)}
    ones_row = np.ones((1, P), np.float16)
    cores = list(range(C))

    def xT_of(c):
        base, ln = shard_base(c), shard_len(c)
        xt = np.zeros((P, NB * P), np.float16)
        xt[:, :ln] = x16[base:base + ln].T
        return xt

    # ---------------- L1: q/k/v tables
    use_bias = any(float(np.abs(np.asarray(v, np.float32)).max()) > 0
                   for v in (bq, bk, bv))
    nc1 = build_l1(with_bias=use_bias)
    in1 = [dict(
        xT=xT_of(c),
        wqkv=np.concatenate([w16["Wq"].T, w16["Wk"].T, w16["Wv"].T],
                            axis=1).copy(),
        bqkv=np.stack([b16["bq"], b16["bk"], b16["bv"]], axis=1).copy())
        for c in cores]
    r1 = run_bass_kernel_spmd(nc1, in1, core_ids=cores)

    tables = {}
    for i, name in enumerate(("q_sh", "k_sh", "v_sh")):
        full = np.zeros((N, F), np.float16)
        for c in cores:
            base, ln = shard_base(c), shard_len(c)
            # qkv_sh is [f_out, (b, t, node)] -> rows = node, cols = f
            sh = r1.results[c]["qkv_sh"].reshape(P, NB, 3, P)[:, :, i, :] \
                .transpose(1, 2, 0).reshape(NB * P, P)
            full[base:base + ln] = sh[:ln]
        tables[name] = full

    # ---------------- L2: src phase -> per-edge exp + per-node recip
    eids = np.arange(E, dtype=np.int64)
    cmap2 = compute_cmap(src, dst)
    plans2 = []
    for c in cores:
        base, ln = shard_base(c), shard_len(c)
        m = (src >= base) & (src < base + ln)
        plans2.append(CorePlan(cmap2, c, src[m], dst[m], eids[m]))

    nc2 = build_l2(cmap2)
    in2 = []
    for c in cores:
        base, ln = shard_base(c), shard_len(c)
        # q table in packed (block, pos) layout for on-chip expansion
        pb, pp = cmap2.perm_block[c], cmap2.perm_pos[c]
        idx = np.zeros(NB * P, np.int64)
        idx[pb * P + pp] = np.arange(ln) + base
        qsh_in = np.ascontiguousarray(
            tables["q_sh"][idx].reshape(NB, P, F).transpose(1, 0, 2)
            .reshape(P, NB * F))
        in2.append(dict(
            qexp=plans2[c].expand_rows(tables["q_sh"], plans2[c].slot_key),
            kexp=plans2[c].expand_rows(tables["k_sh"], plans2[c].slot_gidx),
            q_sh=qsh_in,
            S_st=plans2[c].onehot_stream(False),
            ST_st=plans2[c].onehot_stream(),
            st_idx=plans2[c].loc_idx()))
    r2 = run_bass_kernel_spmd(nc2, in2, core_ids=cores)

    exp_edge = np.zeros((E, H), np.float16)
    rec_full = np.zeros((N, H), np.float16)
    for c in cores:
        pl = plans2[c]
        exp_flat = r2.results[c]["exp_out"].reshape(P, cmap2.nch, H) \
            .transpose(1, 0, 2).reshape(cmap2.nslots, H)
        real = pl.slot_edge >= 0
        exp_edge[pl.slot_edge[real]] = exp_flat[real]
        base, ln = shard_base(c), shard_len(c)
        rsh = r2.results[c]["rec_out"].reshape(P, NB, H)   # [pos, block, h]
        pb, pp = cmap2.perm_block[c], cmap2.perm_pos[c]
        rec_full[base:base + ln] = rsh[pp, pb, :]

    # ---------------- L3: dest phase -> weights, aggregation, projection
    cmap3 = compute_cmap(dst, src)
    plans3 = []
    for c in cores:
        base, ln = shard_base(c), shard_len(c)
        m = (dst >= base) & (dst < base + ln)
        plans3.append(CorePlan(cmap3, c, dst[m], src[m], eids[m]))

    nc3 = build_l3(cmap3)
    in3 = []
    for c in cores:
        pl = plans3[c]
        exp_slots = np.zeros((cmap3.nslots, H), np.float16)
        real = pl.slot_edge >= 0
        exp_slots[real] = exp_edge[pl.slot_edge[real]]
        exp_in = np.ascontiguousarray(
            exp_slots.reshape(cmap3.nch, P, H).transpose(1, 0, 2)
            .reshape(P, cmap3.nch * H))
        in3.append(dict(
            vexp=pl.expand_rows(tables["v_sh"], pl.slot_gidx),
            TT_st=pl.onehot_stream(),
            tt_idx=pl.loc_idx(),
            exp_in=exp_in,
            rexp_in=pl.expand_rows(rec_full, pl.slot_gidx),
            WoT=w16["Wo"].T.copy(),
            bo_r=b16["bo"].reshape(1, P), ones=ones_row))
    r3 = run_bass_kernel_spmd(nc3, in3, core_ids=cores)

    out = np.zeros((N, F), np.float32)
    for c in cores:
        base, ln = shard_base(c), shard_len(c)
        # outT is [fo, (block, pos)] -> rows = packed slot, cols = fo
        ot = r3.results[c]["outT"].reshape(P, NB * P).T
        pb, pp = cmap3.perm_block[c], cmap3.perm_pos[c]
        out[base:base + ln] = ot[pb * P + pp].astype(np.float32)
    return out


# revision 94
# speedup vs baseline: 2.1696x; 1.0050x over previous
"""Trainium2 Bass kernel for nn_EnhancedReflectiveCognitiveGraph (GNN edge-softmax attention).

Math (see reference):
  q/k/v = x @ W{q,k,v}.T + b ; per-edge scores s_e = <q[src_e], k[dest_e]>_head / 4
  softmax over edges sharing src (max-subtraction skipped: scores ~ N(0,1) so
  exp never overflows in fp16 and the weights are mathematically identical;
  the 1/sqrt(d) scale is folded into Wq/bq on the host)
  agg[dest] += exp_e * recip[src_e] * v[src_e] ; out = agg @ Wo.T + bo

Device strategy (8 cores, node-range sharding, three SPMD NEFF launches):
  L1 (proj): each core computes q/k/v (fp16) for its node shard; the host
      keeps the full q/k/v tables (pure relayout).
  L2 (src phase): core c owns edges with src in its shard, laid out in
      128-edge chunks grouped per src block (per-block chunk count k[b] =
      max over cores, so one program serves all 8; the host bin-packs nodes
      into blocks to minimize chunk padding).  q and k rows arrive as
      host-pre-gathered contiguous streams (qexp/kexp, full-bandwidth DMA —
      the host gather is pure relayout of device-computed tables); per-edge
      scores via one fp16 2x multiply + a tree of fp16 adds over the head
      dim; exp (Act); per-src-node segment sums via PE matmuls with one-hot
      matrices (S^T, mostly streamed fp8, a tuned fraction built on DVE via
      iota/is_equal); clamped reciprocal.  Outputs per-edge exp and
      per-node recip.
  L3 (dest phase): core c owns edges with dest in its shard.  v rows arrive
      host-pre-gathered (vexp); the host also permutes exp and gathers recip
      rows per edge (relayout only) so the device computes the softmax
      weights w = exp * recip (fp16 2x) and wv = w * v, scatter-added into
      per-dest-block agg via PE matmuls with streamed one-hots (T^T), then
      the output projection.  agg is complete locally (dest-sharded):
      no collectives and no racy HBM scatter-adds anywhere.
  Host between launches does pure relayout (gather/permute/pad/zero/cast-up).
"""

import math
import ml_dtypes
import numpy as np

import concourse.bacc as bacc
import concourse.mybir as mybir
import concourse.tile as tile
from concourse.bass_utils import run_bass_kernel_spmd

# ---------------------------------------------------------------- constants
N = 50000
E = 600000
F = 128
H = 8
Dh = 16
P = 128
C = 8                     # cores
SH = 6272                 # nodes per core, cores 0-6 (49 blocks); core 7: 6096
NB = 49                   # blocks per shard (core 7 block 48 is partial)
SBV2 = 32                 # chunks per value-stream DMA batch, L2 (fp16)
SBV3 = 36                 # chunks per value-stream DMA batch, L3 (fp16)
SBO = 64                  # chunks per one-hot stream DMA batch (fp8)
SBO2 = 32                 # chunks per one-hot batch when hybrid (stream/build)
OH_BUILD_L2 = 0.0         # fraction of one-hot batches built on DVE in L2
OH_BUILD_L3 = 0.15        # fraction built on DVE in L3
POOL_MULT_L2 = 0.0        # fraction of L2 mult blocks on Pool
POOL_MULT_L3 = 0.0        # fraction of L3 wv-mult blocks on Pool
QS_BLOCKS = 44            # L2 blocks whose q is expanded on-chip (prefix)
QE_CB = 6                 # chunks per qe PSUM batch in the on-chip q path
ESPL = 12                 # blocks per exp/rec output write group
F16 = mybir.dt.float16
F8 = mybir.dt.float8e4
F32 = mybir.dt.float32


def shard_base(c):
    return c * SH


def shard_len(c):
    return min(N, (c + 1) * SH) - c * SH


# ---------------------------------------------------------------- host prep
class ChunkMap:
    """Uniform chunk structure shared by all cores for one phase.

    Chunks (128 slots each) are grouped per key-block: k[b] chunks for block
    b, sized so every core's edges fit.  chunk -> block is data-independent;
    only slot contents differ per core."""

    def __init__(self, ks):
        self.ks = list(ks)
        self.chunks = [b for b in range(NB) for _ in range(self.ks[b])]
        self.block_start = np.cumsum([0] + self.ks).tolist()
        self.nch = len(self.chunks)
        self.nslots = self.nch * P
        self.kmax = max(self.ks)


def _pack_blocks(deg):
    """Pack nodes (by per-phase degree) into NB blocks of <=128 nodes so all
    but a few overflow blocks stay under 12*128 edges: snake-deal by degree
    (near-equal sums), then swap high-degree nodes into the overflow blocks.
    Returns (block, pos) per node and per-block edge sums."""
    ln = len(deg)
    order = np.argsort(-deg, kind="stable")
    members = [[] for _ in range(NB)]
    i = 0
    fwd = True
    while i < ln:
        for b in (range(NB) if fwd else range(NB - 1, -1, -1)):
            if i >= ln:
                break
            members[b].append(order[i])
            i += 1
        fwd = not fwd
    for b in range(NB):
        members[b].sort(key=lambda n: -deg[n])   # desc degree within block
    sums = np.array([int(deg[m].sum()) if len(m) else 0
                     for m in [np.array(mm, np.int64) for mm in members]])
    target = 12 * P
    excess = int(deg.sum()) - NB * target
    m_over = (excess + P - 1) // P + 1 if excess > 0 else 0
    if m_over:
        over = list(np.argsort(sums)[-m_over:])
        over_set = set(over)
        for b in range(NB):
            if b in over_set:
                continue
            while sums[b] > target:
                u = members[b][0]          # largest-degree node in b
                best = None
                for o in over:
                    v = members[o][-1]     # smallest-degree node in o
                    if deg[v] < deg[u] and (best is None
                                            or sums[o] < sums[best[0]]):
                        best = (o, v)      # fill the emptiest overflow block
                if best is None:
                    break
                o, v = best
                members[b].remove(u)
                members[o].remove(v)
                members[b].append(v)
                members[o].insert(0, u)
                members[b].sort(key=lambda n: -deg[n])
                sums[b] += deg[v] - deg[u]
                sums[o] += deg[u] - deg[v]
    # order blocks by edge sum so chunk counts align across cores
    order_b = np.argsort(sums, kind="stable")
    blk_of = np.zeros(ln, np.int64)
    pos = np.zeros(ln, np.int64)
    for newb, b in enumerate(order_b):
        idx = np.array(members[b], np.int64)
        blk_of[idx] = newb
        pos[idx] = np.arange(len(idx))
    return blk_of, pos, sums[order_b]


def compute_cmap(key, other):
    """Per-block chunk counts (max over cores) for one phase, with host-side
    node->block packing (pure relabeling) to minimize chunk padding."""
    need = np.ones(NB, dtype=np.int64)
    perm_block, perm_pos = [], []
    for c in range(C):
        base, ln = shard_base(c), shard_len(c)
        m = (key >= base) & (key < base + ln)
        deg = np.bincount(key[m] - base, minlength=ln)[:ln]
        blk_of, pos, sums = _pack_blocks(deg)
        perm_block.append(blk_of)
        perm_pos.append(pos)
        need = np.maximum(need, (sums + P - 1) // P)
    cm = ChunkMap(need.tolist())
    cm.perm_block = perm_block
    cm.perm_pos = perm_pos
    return cm


class CorePlan:
    """Per-core slot contents for one phase.  `key` = node defining the block
    (src for L2, dest for L3); `other` = the opposite endpoint (indexes the
    host-side gather tables)."""

    def __init__(self, cmap, core, key, other, edge_ids):
        base = shard_base(core)
        pb = cmap.perm_block[core]
        pp = cmap.perm_pos[core]
        self.slot_local = np.full(cmap.nslots, -1, np.int64)
        self.slot_key = np.zeros(cmap.nslots, np.int64)
        self.slot_gidx = np.zeros(cmap.nslots, np.int64)
        self.slot_edge = np.full(cmap.nslots, -1, np.int64)
        block = pb[key - base]
        loc = pp[key - base]
        for b in range(NB):
            m = block == b
            cnt = int(m.sum())
            if cnt == 0:
                continue
            assert cnt <= cmap.ks[b] * P
            s0 = cmap.block_start[b] * P
            self.slot_local[s0:s0 + cnt] = loc[m]
            self.slot_key[s0:s0 + cnt] = key[m]
            self.slot_gidx[s0:s0 + cnt] = other[m]
            self.slot_edge[s0:s0 + cnt] = edge_ids[m]
        self.cmap = cmap

    def onehot_stream(self, transposed=True):
        """One-hot [128, nch*128] fp8; chunk c at cols c*128:(c+1)*128.
        transposed=True: S^T [slot(p), key_local] (seg-sum lhsT);
        transposed=False: S [key_local(p), slot] (expansion lhsT).
        Dummy slots are all-zero rows/columns."""
        cm = self.cmap
        out = np.zeros((P, cm.nch * P), dtype=ml_dtypes.float8_e4m3)
        loc = self.slot_local
        sl_all = np.arange(cm.nslots)
        valid = loc >= 0
        ch = sl_all // P
        row = sl_all % P
        if transposed:
            out[row[valid], ch[valid] * P + loc[valid]] = 1.0
        else:
            out[loc[valid], ch[valid] * P + row[valid]] = 1.0
        return out

    def expand_rows(self, table, idx):
        """Host gather: [128, nch*rowlen] stream, slot (c, p) holds
        table[idx[c*128+p]] (pure relayout of a device-computed table)."""
        cm = self.cmap
        rowlen = table.shape[1]
        g = table[idx.reshape(cm.nch, P)]          # [nch, P, rowlen]
        return np.ascontiguousarray(
            g.transpose(1, 0, 2).reshape(P, cm.nch * rowlen))

    def loc_idx(self):
        """[128, nch] int16: slot_local per (slot-partition, chunk); -1 for
        dummy slots (matches nothing when one-hots are built on-chip)."""
        return np.ascontiguousarray(
            self.slot_local.reshape(self.cmap.nch, P).T.astype(np.int16))


# ---------------------------------------------------------------- L1: q/k/v projections
def build_l1(with_bias=False):
    nc = bacc.Bacc("TRN2", target_bir_lowering=False, num_devices=C)
    xT = nc.dram_tensor("xT", [P, NB * P], F16, kind="ExternalInput")
    wqkv = nc.dram_tensor("wqkv", [P, 3 * P], F16, kind="ExternalInput")
    bqkv = nc.dram_tensor("bqkv", [P, 3], F16, kind="ExternalInput")
    # q/k/v transposed [f_out, node], interleaved per block; the host
    # de-interleaves and transposes (pure relayout)
    qkv_sh = nc.dram_tensor("qkv_sh", [P, NB * 3 * P], F16,
                            kind="ExternalOutput")

    with tile.TileContext(nc) as tc:
        with tc.tile_pool(name="const", bufs=1) as cpool, \
             tc.tile_pool(name="psum", bufs=4, space="PSUM") as ppool:
            w_sb = cpool.tile([P, 3 * P], F16, tag="w")
            nc.sync.dma_start(w_sb[:], wqkv[:])
            b_sb = cpool.tile([P, 3], F16, tag="b")
            nc.sync.dma_start(b_sb[:], bqkv[:])
            xt = cpool.tile([P, NB * P], F16, tag="xT")
            LD = 7

            def load_x(slice_i):
                b0 = slice_i * LD
                if b0 >= NB:
                    return
                n = min(LD, NB - b0) * P
                nc.gpsimd.dma_start(xt[:, b0 * P:b0 * P + n],
                                    xT[:, b0 * P:b0 * P + n])

            # two slices ahead; the rest interleave with output writes so
            # the serial DMA engine alternates input/output instead of
            # front-loading all input
            load_x(0)
            load_x(1)
            osb = cpool.tile([P, NB * 3 * P], F16, tag="osb")
            for b in range(NB):
                if b % LD == 0 and b > 0:
                    load_x(b // LD + 1)
                # out[f_out, node] = W[f_in, f_out]^T @ xT[f_in, node]:
                # bias becomes a per-partition scalar, no bias matmul
                ps = ppool.tile([P, 3 * P], F32, tag="proj")
                for t in range(3):
                    nc.tensor.matmul(ps[:, t * P:(t + 1) * P],
                                     lhsT=w_sb[:, t * P:(t + 1) * P],
                                     rhs=xt[:, b * P:(b + 1) * P],
                                     start=True, stop=True)
                    if with_bias:
                        dst = osb[:, (b * 3 + t) * P:(b * 3 + t + 1) * P]
                        if (b * 3 + t) % 2:
                            nc.vector.tensor_tensor(
                                out=dst, in0=ps[:, t * P:(t + 1) * P],
                                in1=b_sb[:, t:t + 1].broadcast_to([P, P]),
                                op=mybir.AluOpType.add)
                        else:
                            nc.scalar.activation(
                                out=dst, in_=ps[:, t * P:(t + 1) * P],
                                func=mybir.ActivationFunctionType.Identity,
                                bias=b_sb[:, t:t + 1])
                if not with_bias:
                    # biases are zero: one whole-block copy, alternating
                    (nc.vector.tensor_copy if b % 2 else nc.scalar.copy)(
                        osb[:, b * 3 * P:(b + 1) * 3 * P], ps[:])
                # write in 4-block groups from the Pool queue so output DMA
                # overlaps compute from block 3 onward
                WD = 4
                if b % WD == WD - 1 or b == NB - 1:
                    b0 = b // WD * WD
                    n = (b - b0 + 1) * 3 * P
                    nc.sync.dma_start(qkv_sh[:, b0 * 3 * P:b0 * 3 * P + n],
                                      osb[:, b0 * 3 * P:b0 * 3 * P + n])
    nc.compile()
    return nc


# ---------------------------------------------------------------- streaming helper
def make_streamer(nc, spool, nch, sb, first=None, last=None, lstep=8):
    """Stream [P, nch*rowlen] dram in sb-chunk batches.  A smaller first
    batch (`first`) shortens the pipeline ramp; splitting the final `last`
    chunks into `lstep`-sized batches shortens the drain tail."""
    bounds = [0]
    x = 0
    while x < nch:
        if x == 0 and first:
            step = first
        elif last and x >= nch - last:
            step = lstep
        else:
            step = sb
        x = min(nch, x + step)
        bounds.append(x)

    def stream_tile(tiles, dram, ci, dt, rowlen=P):
        import bisect
        i = bisect.bisect_right(bounds, ci) - 1
        b0, hi = bounds[i], bounds[i + 1]
        if b0 not in tiles:
            t = spool.tile([P, sb * rowlen], dt, tag=dram.name,
                           name=f"strm_{dram.name}_{b0}")
            n = (hi - b0) * rowlen
            nc.sync.dma_start(t[:, :n], dram[:, b0 * rowlen:b0 * rowlen + n])
            tiles[b0] = t
        return tiles[b0], b0, hi
    return stream_tile


def make_onehot_source(nc, pool, nch, sb, dram, idx_sb, iota_t, build_frac):
    """One-hot chunk source: batches are either DMA-streamed from `dram` or
    built on DVE (is_equal is not in the Pool engine's ISA) from compact
    local indices via iota/is_equal.  `build_frac` of batches are built,
    spread evenly."""
    tiles = {}
    nbatch = (nch + sb - 1) // sb
    built = set(i for i in range(nbatch)
                if int(i * build_frac) != int((i + 1) * build_frac))

    def get(ci):
        b0 = ci // sb * sb
        hi = min(b0 + sb, nch)
        if b0 not in tiles:
            n = hi - b0
            t = pool.tile([P, sb * P], F8, tag=dram.name,
                          name=f"oh_{dram.name}_{b0}")
            if (b0 // sb) in built:
                nc.vector.tensor_tensor(
                    out=t[:, :n * P].rearrange("p (c q) -> p c q", q=P),
                    in0=iota_t[:, None, :].broadcast_to([P, n, P]),
                    in1=idx_sb[:, b0:hi][:, :, None].broadcast_to([P, n, P]),
                    op=mybir.AluOpType.is_equal)
            else:
                nc.sync.dma_start(t[:, :n * P], dram[:, b0 * P:hi * P])
            tiles[b0] = t
        return tiles[b0], b0, hi
    return get


# ---------------------------------------------------------------- L2: src phase
def build_l2(cmap, qs_blocks=None):
    """qs_blocks: the first qs_blocks blocks expand q ON-CHIP (PE one-hot
    matmul from a resident per-shard q table + Act PSUM->fp16 copy) instead
    of reading the host-expanded qexp stream — trades idle Act/PE cycles for
    DMA bytes."""
    if qs_blocks is None:
        qs_blocks = QS_BLOCKS
    nch, kmax = cmap.nch, cmap.kmax
    nc = bacc.Bacc("TRN2", target_bir_lowering=False, num_devices=C)
    qexp = nc.dram_tensor("qexp", [P, nch * P], F16, kind="ExternalInput")
    kexp = nc.dram_tensor("kexp", [P, nch * P], F16, kind="ExternalInput")
    q_sh = nc.dram_tensor("q_sh", [P, NB * P], F16, kind="ExternalInput")
    S_st = nc.dram_tensor("S_st", [P, nch * P], F8, kind="ExternalInput")
    ST_st = nc.dram_tensor("ST_st", [P, nch * P], F8, kind="ExternalInput")
    st_idx = nc.dram_tensor("st_idx", [P, nch], mybir.dt.int16,
                            kind="ExternalInput")
    exp_out = nc.dram_tensor("exp_out", [P, nch * H], F16, kind="ExternalOutput")
    rec_out = nc.dram_tensor("rec_out", [P, NB * H], F16, kind="ExternalOutput")
    CB = QE_CB                 # chunks per qe PSUM batch (2 banks x2 bufs)

    with tile.TileContext(nc) as tc:
        with tc.tile_pool(name="resident", bufs=1) as rpool, \
             tc.tile_pool(name="ostream", bufs=3) as spool, \
             tc.tile_pool(name="vstream", bufs=4) as vpool, \
             tc.tile_pool(name="work", bufs=3) as wpool, \
             tc.tile_pool(name="seg_psum", bufs=2, space="PSUM") as gpsum, \
             tc.tile_pool(name="qe_psum", bufs=2, space="PSUM") as qpsum:
            exp_sb = rpool.tile([P, nch * H], F16, tag="exp_sb")
            rec_sb = rpool.tile([P, NB * H], F16, tag="rec_sb")
            idx_sb = iota_t = None
            if OH_BUILD_L2 > 0:
                idx_sb = rpool.tile([P, nch], mybir.dt.int16, tag="idx_sb")
                nc.sync.dma_start(idx_sb[:], st_idx[:])
                iota_t = rpool.tile([P, P], mybir.dt.int16, tag="iota")
                nc.gpsimd.iota(iota_t[:], pattern=[[1, P]], base=0,
                               channel_multiplier=0)
            st_src = make_onehot_source(nc, spool, nch, SBO2, ST_st,
                                        idx_sb, iota_t, OH_BUILD_L2)
            vstream = make_streamer(nc, vpool, nch, SBV2, first=8)
            sstream = make_streamer(nc, spool, nch, SBO2)
            q_tiles, k_tiles, s_tiles = {}, {}, {}
            qsb = None
            if qs_blocks:
                qsb = rpool.tile([P, NB * P], F16, tag="q_sh_sb")
                # staged load: early blocks' slices land first so the qe
                # pipeline starts immediately
                edges = [0, 2, 6, 14, 28, qs_blocks]
                for lo, hi in zip(edges, edges[1:]):
                    hi = min(hi, qs_blocks)
                    if hi > lo:
                        # Pool queue: interleaves with SP-queue stream loads
                        nc.gpsimd.dma_start(qsb[:, lo * P:hi * P],
                                            q_sh[:, lo * P:hi * P])

            esplit = [cmap.block_start[min(b0 + ESPL, NB)]
                      for b0 in range(0, NB, ESPL)]

            for b in range(NB):
                kb = cmap.ks[b]
                s0 = cmap.block_start[b]
                qk = wpool.tile([P, kmax * P], F16, tag="qk")
                if b < qs_blocks:
                    # on-chip q expansion: PE matmuls + scaled Act copy
                    qe16 = wpool.tile([P, kmax * P], F16, tag="qe16")
                    ci = s0
                    while ci < s0 + kb:
                        cn = min(CB, s0 + kb - ci)
                        qe_ps = qpsum.tile([P, CB * P], F32, tag="qe",
                                           name=f"qe_{ci}")
                        for j in range(cn):
                            st, sb0, _ = sstream(s_tiles, S_st, ci + j, F8)
                            off = ci + j - sb0
                            nc.tensor.matmul(
                                qe_ps[:, j * P:(j + 1) * P],
                                lhsT=st[:, off * P:(off + 1) * P],
                                rhs=qsb[:, b * P:(b + 1) * P],
                                start=True, stop=True)
                        nc.scalar.activation(
                            out=qe16[:, (ci - s0) * P:(ci - s0 + cn) * P],
                            in_=qe_ps[:, :cn * P],
                            func=mybir.ActivationFunctionType.Copy,
                            scale=1.0)
                        ci += cn
                # qk = q * kexp (DVE 2x), split at stream-batch boundaries
                ci = s0
                while ci < s0 + kb:
                    kt, b0, bhi = vstream(k_tiles, kexp, ci, F16)
                    cj = min(s0 + kb, bhi)
                    n = cj - ci
                    if b < qs_blocks:
                        nc.vector.tensor_mul(
                            qk[:, (ci - s0) * P:(ci - s0 + n) * P],
                            qe16[:, (ci - s0) * P:(ci - s0 + n) * P],
                            kt[:, (ci - b0) * P:(ci - b0 + n) * P])
                    else:
                        qt, qb0, _ = vstream(q_tiles, qexp, ci, F16)
                        nc.vector.tensor_mul(
                            qk[:, (ci - s0) * P:(ci - s0 + n) * P],
                            qt[:, (ci - qb0) * P:(ci - qb0 + n) * P],
                            kt[:, (ci - b0) * P:(ci - b0 + n) * P])
                    ci = cj
                # tree-reduce over head dim (DVE 2x fp16 adds)
                a1 = wpool.tile([P, kmax * P // 2], F16, tag="a1")
                qk4 = qk[:, :kb * P].rearrange("p (c h d) -> p c h d", h=H, d=Dh)
                nc.vector.tensor_add(
                    out=a1[:, :kb * P // 2].rearrange("p (c h d) -> p c h d", h=H, d=8),
                    in0=qk4[:, :, :, 0:8], in1=qk4[:, :, :, 8:16])
                a2 = wpool.tile([P, kmax * P // 4], F16, tag="a2")
                a14 = a1[:, :kb * P // 2].rearrange("p (c h d) -> p c h d", h=H, d=8)
                nc.vector.tensor_add(
                    out=a2[:, :kb * P // 4].rearrange("p (c h d) -> p c h d", h=H, d=4),
                    in0=a14[:, :, :, 0:4], in1=a14[:, :, :, 4:8])
                a3 = wpool.tile([P, kmax * P // 8], F16, tag="a3")
                a24 = a2[:, :kb * P // 4].rearrange("p (c h d) -> p c h d", h=H, d=4)
                nc.vector.tensor_add(
                    out=a3[:, :kb * P // 8].rearrange("p (c h d) -> p c h d", h=H, d=2),
                    in0=a24[:, :, :, 0:2], in1=a24[:, :, :, 2:4])
                sc = wpool.tile([P, kmax * H], F16, tag="sc")
                a34 = a3[:, :kb * P // 8].rearrange("p (c h d) -> p c h d", h=H, d=2)
                nc.vector.tensor_add(
                    out=sc[:, :kb * H].rearrange("p (c h) -> p c h", h=H)[:, :, :, None],
                    in0=a34[:, :, :, 0:1], in1=a34[:, :, :, 1:2])
                # exp (Act) straight into the resident output tile
                nc.scalar.activation(
                    out=exp_sb[:, s0 * H:(s0 + kb) * H], in_=sc[:, :kb * H],
                    func=mybir.ActivationFunctionType.Exp, scale=1.0)
                # segment sums over slots per src node (PE, accumulate)
                seg_ps = gpsum.tile([P, H], F32, tag="seg")
                for j in range(kb):
                    st, sb0, _ = st_src(s0 + j)
                    off = s0 + j - sb0
                    nc.tensor.matmul(
                        seg_ps[:],
                        lhsT=st[:, off * P:(off + 1) * P],
                        rhs=exp_sb[:, (s0 + j) * H:(s0 + j + 1) * H],
                        start=(j == 0), stop=(j == kb - 1))
                # clamped reciprocal: zero-degree rows give a finite value
                # (cancelled downstream); real rows have seg >> 1e-4
                s2 = wpool.tile([P, H], F32, tag="s2")
                nc.vector.tensor_scalar_max(s2[:], seg_ps[:], 1e-4)
                rec_raw = wpool.tile([P, H], F32, tag="rec_raw")
                nc.vector.reciprocal(rec_raw[:], s2[:])
                nc.scalar.copy(rec_sb[:, b * H:(b + 1) * H], rec_raw[:])
                # stream finished exp/rec columns out in 6-block groups
                hi = cmap.block_start[b + 1]
                if hi in esplit:
                    lo = esplit[esplit.index(hi) - 1] if esplit.index(hi) else 0
                    # final split rides the faster sync path (tail: no
                    # input prefetch left to block)
                    eng = nc.sync if b == NB - 1 else nc.gpsimd
                    eng.dma_start(exp_out[:, lo * H:hi * H],
                                  exp_sb[:, lo * H:hi * H])
                    lob = b // ESPL * ESPL
                    eng.dma_start(rec_out[:, lob * H:(b + 1) * H],
                                  rec_sb[:, lob * H:(b + 1) * H])
    nc.compile()
    return nc


# ---------------------------------------------------------------- L3: dest phase
def build_l3(cmap):
    nch, kmax = cmap.nch, cmap.kmax
    nc = bacc.Bacc("TRN2", target_bir_lowering=False, num_devices=C)
    vexp = nc.dram_tensor("vexp", [P, nch * P], F16, kind="ExternalInput")
    TT_st = nc.dram_tensor("TT_st", [P, nch * P], F8, kind="ExternalInput")
    tt_idx = nc.dram_tensor("tt_idx", [P, nch], mybir.dt.int16,
                            kind="ExternalInput")
    exp_in = nc.dram_tensor("exp_in", [P, nch * H], F16, kind="ExternalInput")
    rexp_in = nc.dram_tensor("rexp_in", [P, nch * H], F16, kind="ExternalInput")
    WoT = nc.dram_tensor("WoT", [P, P], F16, kind="ExternalInput")
    bo_r = nc.dram_tensor("bo_r", [1, P], F16, kind="ExternalInput")
    ones = nc.dram_tensor("ones", [1, P], F16, kind="ExternalInput")
    outT = nc.dram_tensor("outT", [P, NB * P], F16, kind="ExternalOutput")

    with tile.TileContext(nc) as tc:
        with tc.tile_pool(name="resident", bufs=1) as rpool, \
             tc.tile_pool(name="ostream", bufs=3) as spool, \
             tc.tile_pool(name="vstream", bufs=4) as vpool, \
             tc.tile_pool(name="work", bufs=3) as wpool, \
             tc.tile_pool(name="agg_psum", bufs=2, space="PSUM") as apsum, \
             tc.tile_pool(name="out_psum", bufs=2, space="PSUM") as opsum:
            idx_sb = iota_t = None
            if OH_BUILD_L3 > 0:
                idx_sb = rpool.tile([P, nch], mybir.dt.int16, tag="idx_sb")
                nc.sync.dma_start(idx_sb[:], tt_idx[:])
                iota_t = rpool.tile([P, P], mybir.dt.int16, tag="iota")
                nc.gpsimd.iota(iota_t[:], pattern=[[1, P]], base=0,
                               channel_multiplier=0)
            tt_src = make_onehot_source(nc, spool, nch, SBO2, TT_st,
                                        idx_sb, iota_t, OH_BUILD_L3)
            vstream = make_streamer(nc, vpool, nch, SBV3, first=8, last=24)
            v_tiles = {}
            # prefetch the first value/one-hot batches before anything else
            vstream(v_tiles, vexp, 0, F16)
            tt_src(0)
            w_all = rpool.tile([P, nch * H], F16, tag="w_all")
            for lo_b, hi_b in ((0, 6), (6, 16), (16, 27), (27, 38), (38, NB)):
                lo, hi = cmap.block_start[lo_b], cmap.block_start[hi_b]
                nc.vector.tensor_mul(
                    w_all[:, lo * H:hi * H],
                    nc_stream_slice(nc, exp_in, rpool, lo, hi),
                    nc_stream_slice(nc, rexp_in, rpool, lo, hi))
            wo_sb = rpool.tile([P, P], F16, tag="wo")
            nc.sync.dma_start(wo_sb[:], WoT[:])
            bo_sb = rpool.tile([1, P], F16, tag="bo")
            nc.sync.dma_start(bo_sb[:], bo_r[:])
            ones_sb = rpool.tile([1, P], F16, tag="ones")
            nc.sync.dma_start(ones_sb[:], ones[:])
            osb = rpool.tile([P, NB * P], F16, tag="osb")

            for b in range(NB):
                kb = cmap.ks[b]
                s0 = cmap.block_start[b]
                # wv = w * v: per-block strategy balances Pool/Act/DVE.
                # 'act': Act broadcasts w, DVE multiplies in 2x mode;
                # 'dve': single DVE 1x broadcast-multiply;
                # 'pool': single Pool broadcast-multiply.
                if int(b * POOL_MULT_L3) != int((b + 1) * POOL_MULT_L3):
                    strat = 'pool'
                else:
                    strat = 'act' if b % 2 == 0 else 'dve'
                wv = wpool.tile([P, kmax * P], F16, tag="wv")
                if strat == 'act':
                    # Act broadcast of w, then DVE 2x multiply
                    erep = wpool.tile([P, kmax * P], F16, tag="erep")
                    nc.scalar.copy(
                        erep[:, :kb * P].rearrange("p (c h d) -> p c h d", h=H, d=Dh),
                        w_all[:, s0 * H:(s0 + kb) * H]
                        .rearrange("p (c h) -> p c h", h=H)[:, :, :, None]
                        .broadcast_to([P, kb, H, Dh]))
                ci = s0
                while ci < s0 + kb:
                    vt, b0, bhi = vstream(v_tiles, vexp, ci, F16)
                    cj = min(s0 + kb, bhi)
                    n = cj - ci
                    if strat == 'act':
                        nc.vector.tensor_mul(
                            wv[:, (ci - s0) * P:(ci - s0 + n) * P],
                            vt[:, (ci - b0) * P:(ci - b0 + n) * P],
                            erep[:, (ci - s0) * P:(ci - s0 + n) * P])
                    else:
                        eng = (nc.gpsimd.tensor_tensor if strat == 'pool'
                               else nc.vector.tensor_tensor)
                        eng(out=wv[:, (ci - s0) * P:(ci - s0 + n) * P]
                            .rearrange("p (c h d) -> p c h d", h=H, d=Dh),
                            in0=vt[:, (ci - b0) * P:(ci - b0 + n) * P]
                            .rearrange("p (c h d) -> p c h d", h=H, d=Dh),
                            in1=w_all[:, ci * H:(ci + n) * H]
                            .rearrange("p (c h) -> p c h", h=H)[:, :, :, None]
                            .broadcast_to([P, n, H, Dh]),
                            op=mybir.AluOpType.mult)
                    ci = cj
                # scatter-add into aggT[f, dest_local] (PE, accumulate)
                agg_ps = apsum.tile([P, P], F32, tag="agg")
                for j in range(kb):
                    tt, sb0, _ = tt_src(s0 + j)
                    off = s0 + j - sb0
                    nc.tensor.matmul(
                        agg_ps[:],
                        lhsT=wv[:, j * P:(j + 1) * P],
                        rhs=tt[:, off * P:(off + 1) * P],
                        start=(j == 0), stop=(j == kb - 1))
                agg16 = wpool.tile([P, P], F16, tag="agg16")
                nc.scalar.copy(agg16[:], agg_ps[:])
                # output projection (PE) and fp16 store
                ops = opsum.tile([P, P], F32, tag="outp")
                nc.tensor.matmul(ops[:], lhsT=wo_sb[:], rhs=agg16[:],
                                 start=True, stop=False)
                nc.tensor.matmul(ops[:], lhsT=bo_sb[:], rhs=ones_sb[:],
                                 start=False, stop=True)
                nc.scalar.copy(osb[:, b * P:(b + 1) * P], ops[:])
                if b % 7 == 6 or b == NB - 1:
                    b0 = b // 7 * 7
                    n = (b - b0 + 1) * P
                    eng = nc.sync if b == NB - 1 else nc.gpsimd
                    eng.dma_start(outT[:, b0 * P:b0 * P + n],
                                  osb[:, b0 * P:b0 * P + n])
    nc.compile()
    return nc


def nc_stream_slice(nc, dram, pool, lo, hi):
    """Load a column slice of a small dram tensor into a fresh tile."""
    t = pool.tile([P, (hi - lo) * H], F16, tag=f"{dram.name}_{lo}",
                  name=f"{dram.name}_{lo}")
    nc.sync.dma_start(t[:], dram[:, lo * H:hi * H])
    return t[:]


# ---------------------------------------------------------------- orchestration
def kernel(node_features, edge_index, Wq, bq, Wk, bk, Wv, bv, Wo, bo):
    node_features = np.asarray(node_features, np.float32)
    edge_index = np.asarray(edge_index)
    src, dst = edge_index[0].astype(np.int64), edge_index[1].astype(np.int64)
    x16 = node_features.astype(np.float16)
    s = 1.0 / math.sqrt(Dh)   # fold score scale into Wq/bq (parameter prep)
    w16 = {"Wq": (np.asarray(Wq, np.float32) * s).astype(np.float16),
           "Wk": np.asarray(Wk, np.float32).astype(np.float16),
           "Wv": np.asarray(Wv, np.float32).astype(np.float16),
           "Wo": np.asarray(Wo, np.float32).astype(np.float16)}
    b16 = {"bq": (np.asarray(bq, np.float32) * s).astype(np.float16),
           "bk": np.asarray(bk, np.float32).astype(np.float16),
           "bv": np.asarray(bv, np.float32).astype(np.float16),
           "bo": np.asarray(bo, np.float32).astype(np.float16)}
    ones_row = np.ones((1, P), np.float16)
    cores = list(range(C))

    def xT_of(c):
        base, ln = shard_base(c), shard_len(c)
        xt = np.zeros((P, NB * P), np.float16)
        xt[:, :ln] = x16[base:base + ln].T
        return xt

    # ---------------- L1: q/k/v tables
    use_bias = any(float(np.abs(np.asarray(v, np.float32)).max()) > 0
                   for v in (bq, bk, bv))
    nc1 = build_l1(with_bias=use_bias)
    in1 = [dict(
        xT=xT_of(c),
        wqkv=np.concatenate([w16["Wq"].T, w16["Wk"].T, w16["Wv"].T],
                            axis=1).copy(),
        bqkv=np.stack([b16["bq"], b16["bk"], b16["bv"]], axis=1).copy())
        for c in cores]
    r1 = run_bass_kernel_spmd(nc1, in1, core_ids=cores)

    tables = {}
    for i, name in enumerate(("q_sh", "k_sh", "v_sh")):
        full = np.zeros((N, F), np.float16)
        for c in cores:
            base, ln = shard_base(c), shard_len(c)
            # qkv_sh is [f_out, (b, t, node)] -> rows = node, cols = f
            sh = r1.results[c]["qkv_sh"].reshape(P, NB, 3, P)[:, :, i, :] \
                .transpose(1, 2, 0).reshape(NB * P, P)
            full[base:base + ln] = sh[:ln]
        tables[name] = full

    # ---------------- L2: src phase -> per-edge exp + per-node recip
    eids = np.arange(E, dtype=np.int64)
    cmap2 = compute_cmap(src, dst)
    plans2 = []
    for c in cores:
        base, ln = shard_base(c), shard_len(c)
        m = (src >= base) & (src < base + ln)
        plans2.append(CorePlan(cmap2, c, src[m], dst[m], eids[m]))

    nc2 = build_l2(cmap2)
    in2 = []
    for c in cores:
        base, ln = shard_base(c), shard_len(c)
        # q table in packed (block, pos) layout for on-chip expansion
        pb, pp = cmap2.perm_block[c], cmap2.perm_pos[c]
        idx = np.zeros(NB * P, np.int64)
        idx[pb * P + pp] = np.arange(ln) + base
        qsh_in = np.ascontiguousarray(
            tables["q_sh"][idx].reshape(NB, P, F).transpose(1, 0, 2)
            .reshape(P, NB * F))
        in2.append(dict(
            qexp=plans2[c].expand_rows(tables["q_sh"], plans2[c].slot_key),
            kexp=plans2[c].expand_rows(tables["k_sh"], plans2[c].slot_gidx),
            q_sh=qsh_in,
            S_st=plans2[c].onehot_stream(False),
            ST_st=plans2[c].onehot_stream(),
            st_idx=plans2[c].loc_idx()))
    r2 = run_bass_kernel_spmd(nc2, in2, core_ids=cores)

    exp_edge = np.zeros((E, H), np.float16)
    rec_full = np.zeros((N, H), np.float16)
    for c in cores:
        pl = plans2[c]
        exp_flat = r2.results[c]["exp_out"].reshape(P, cmap2.nch, H) \
            .transpose(1, 0, 2).reshape(cmap2.nslots, H)
        real = pl.slot_edge >= 0
        exp_edge[pl.slot_edge[real]] = exp_flat[real]
        base, ln = shard_base(c), shard_len(c)
        rsh = r2.results[c]["rec_out"].reshape(P, NB, H)   # [pos, block, h]
        pb, pp = cmap2.perm_block[c], cmap2.perm_pos[c]
        rec_full[base:base + ln] = rsh[pp, pb, :]

    # ---------------- L3: dest phase -> weights, aggregation, projection
    cmap3 = compute_cmap(dst, src)
    plans3 = []
    for c in cores:
        base, ln = shard_base(c), shard_len(c)
        m = (dst >= base) & (dst < base + ln)
        plans3.append(CorePlan(cmap3, c, dst[m], src[m], eids[m]))

    nc3 = build_l3(cmap3)
    in3 = []
    for c in cores:
        pl = plans3[c]
        exp_slots = np.zeros((cmap3.nslots, H), np.float16)
        real = pl.slot_edge >= 0
        exp_slots[real] = exp_edge[pl.slot_edge[real]]
        exp_in = np.ascontiguousarray(
            exp_slots.reshape(cmap3.nch, P, H).transpose(1, 0, 2)
            .reshape(P, cmap3.nch * H))
        in3.append(dict(
            vexp=pl.expand_rows(tables["v_sh"], pl.slot_gidx),
            TT_st=pl.onehot_stream(),
            tt_idx=pl.loc_idx(),
            exp_in=exp_in,
            rexp_in=pl.expand_rows(rec_full, pl.slot_gidx),
            WoT=w16["Wo"].T.copy(),
            bo_r=b16["bo"].reshape(1, P), ones=ones_row))
    r3 = run_bass_kernel_spmd(nc3, in3, core_ids=cores)

    out = np.zeros((N, F), np.float32)
    for c in cores:
        base, ln = shard_base(c), shard_len(c)
        # outT is [fo, (block, pos)] -> rows = packed slot, cols = fo
        ot = r3.results[c]["outT"].reshape(P, NB * P).T
        pb, pp = cmap3.perm_block[c], cmap3.perm_pos[c]
        out[base:base + ln] = ot[pb * P + pp].astype(np.float32)
    return out
